# revision 14
# baseline (speedup 1.0000x reference)
"""Autoregressive GRU on 8 TRN2 NeuronCores — folded fp16 col-tiled design.

Data-parallel: batch B=512 split 64 rows/core; weights replicated; the T=128
sequential loop runs locally per core.

Math (Keras GRU, reset_after=True, gate order [z, r, h]), host-prefused:
  V  = [Wr+Ur | Uh | Wh | Wz+Uz]  [D, 4D]  (steps >= 1; step 0 uses V0)
  PSUM bank order [r | hh | xh | z]:
  r = sig(g_r); p = r*g_hh; q = p + g_xh; hhat = tanh(q); z = sig(g_z)
  h_new = hhat + z*(h - hhat)

Measured: 586443 ns HW exec (vs 906919 ns for the previous f32r/bf16
design on the same harness; 1077896 ns session-start baseline), rel err
1.93e-3 (was 9.6e-3). The z bank is computed as two column-half
accumulation groups in SEPARATE PSUM banks so sigmoid(z_L) and the
tt_L/h_newL chain overlap the z_R half's matmul stream (engine reads of
a bank the PE is writing are fatal; separate banks make the overlap
legal) — worth 36us on top of the single-transpose rebuild. The HAM tail
anchors are five half-array K=128/M=64/N=256 matmuls tied to tail tiles
(ident/hhat/s/z/tt stationaries, hT moving): long enough to make the
tail's free-running HAM window look busy (the re-throttle decision lands
one window LATER, mid-next-stream — traced as streams starting warm at
106ns/pair then dropping to 207ns mid-stream), small enough (~+30
percent PE energy) to stay out of the P0 power state.

Perf structure vs the previous f32r design:
- FOLDED layout: every [64, 512] gate tile becomes [128, 256] by computing
  its two 256-col halves on PE column strips {0-63}/{64-127} via TWO
  CONCURRENT col-tiled matmuls (tile_position (0,0)/(0,64), same hT chunk
  stationary, different V halves). Microbenched: 16 N=512 f32r MMs =
  3796 ns/iter -> 16 concurrent N=256 fp16 pairs = 2195 ns/iter.
  Col tiling requires non-32-bit operands (walrus: f32r rejects dst
  partition 64; f32r×bf16 mixing unsupported) -> both operands fp16.
- fp16 (not bf16): 10 mantissa bits; values are small (|V|<0.2, |h|<~4) so
  no range risk. Numpy-emulated end-to-end rel err: fp16 everywhere 1.9e-3
  vs 9.8e-3 for the old f32r/bf16 design (gate is 2e-2).
- All DVE/ACT tail ops run [128, 256] (full 128 lanes) — 2x the old rate.
- hT rebuild: h_new = hhat + tt is computed first (split L/R halves),
  then ONE full [128,128] PE transpose per half: (h_new[:, 128c:+128])^T
  is exactly [hT chunk c | chunk c+2] in the folded layout (out[c',64f+b]
  = h[b, 256f+c']), full-partition stationary so no quadrant placement
  (transposes of [64, x] slices at partition base 64 hang the device —
  quadrant HW bug). Assembly: per-chunk copies (first chunk on ACT, the
  other on DVE). Next step's matmuls consume hT per 64-col chunk (subtile
  deps), k-order (0,2,1,3), so the stream restarts after the first chunk
  copy lands instead of after the full rebuild.
- trpL/trpR are separate PSUM banks, and each is read by a single copy
  that waits on both its writers (PE-W + engine-R same bank is fatal).
- Output is DMAed as fp16 straight from h_new (no f32 copy; host casts).
- Warm-up preamble of identity transposes flips the PE HAM clock gate to
  K=8/8 before step 0 (PE otherwise starts at 1.2 GHz).
"""

import numpy as np

B, D, T = 512, 512, 128
NCORES = 8
BLOC = B // NCORES  # 64
P = 128
KC = D // P  # 4 K-chunks
GW = 4 * D  # 2048 gate columns: [r | hh | xh | z]
HF = 256  # folded free size (D // 2)

# set by test harness to capture a profile; harmless when False
TRACE = False
TMPDIR = None
LAST = {}


def _prepare_weights(W, U, b):
    """Host-side fusion. Returns (V, V0, bias) in math layout."""
    Wz, Wr, Wh = W[:, :D], W[:, D : 2 * D], W[:, 2 * D :]
    Uz, Ur, Uh = U[:, :D], U[:, D : 2 * D], U[:, 2 * D :]
    V = np.concatenate([Wr + Ur, Uh, Wh, Wz + Uz], axis=1)  # [D, GW]
    V0 = np.concatenate([Ur, Uh, np.zeros_like(Wh), Uz], axis=1)
    b0, b1 = b[0], b[1]
    bias = np.concatenate(
        [b0[D : 2 * D] + b1[D : 2 * D], b1[2 * D :], b0[2 * D :], b0[:D] + b1[:D]]
    )  # [GW], order [r | hh | xh | z]
    return V, V0, bias


def _dev_layout(V):
    # bank-major: V_dev[p, n*(KC*512) + k*512 + c] = V[k*128 + p, n*512 + c]
    # so each gate bank's weights are one contiguous 512KB block and the
    # first stream can start as soon as bank r's DMA lands
    A = V.reshape(KC, P, 4, 512)
    return np.ascontiguousarray(A.transpose(1, 2, 0, 3).reshape(P, KC * GW))


_CACHE = {}

KORDER = (0, 2, 1, 3)  # consume hT L-half chunks (0,2) first
# hT column layout: chunk k lives at cols [CPOS[k], CPOS[k]+64)
CPOS = {0: 0, 2: 64, 1: 128, 3: 192}


def _build(has_bias: bool):
    import concourse.mybir as mybir
    import concourse.tile as tile
    from concourse import bacc
    from concourse.masks import make_identity

    f32 = mybir.dt.float32
    f16 = mybir.dt.float16
    AF = mybir.ActivationFunctionType

    nc = bacc.Bacc(
        "TRN2", target_bir_lowering=False, debug=False, num_devices=NCORES
    )
    v0_d = nc.dram_tensor("v0", [P, KC * GW], f16, kind="ExternalInput").ap()
    v_d = nc.dram_tensor("v", [P, KC * GW], f16, kind="ExternalInput").ap()
    h0_d = nc.dram_tensor("h0", [P, HF], f16, kind="ExternalInput").ap()
    h0T_d = nc.dram_tensor("h0T", [P, HF], f16, kind="ExternalInput").ap()
    if has_bias:
        bias_d = nc.dram_tensor("bias", [P, 4 * HF], f32, kind="ExternalInput").ap()
    out_d = nc.dram_tensor("out", [T, P, HF], f16, kind="ExternalOutput").ap()

    with tile.TileContext(nc) as tc:
        with (
            tc.tile_pool(name="const", bufs=1) as cpool,
            tc.tile_pool(name="state", bufs=2) as spool,
            tc.tile_pool(name="work", bufs=3) as wpool,
            tc.tile_pool(name="gates", bufs=1, space="PSUM") as gpool,
            tc.tile_pool(name="trp", bufs=1, space="PSUM") as trpool,
            tc.tile_pool(name="warm", bufs=1, space="PSUM") as warmpool,
        ):
            v0_sb = cpool.tile([P, KC * GW], f16, tag="v0")
            v_sb = cpool.tile([P, KC * GW], f16, tag="v")
            ident = cpool.tile([P, P], f16, tag="ident")
            h = spool.tile([P, HF], f16, tag="h")
            hT = spool.tile([P, HF], f16, tag="hT")
            nc.sync.dma_start(hT[:], h0T_d[:])
            BK = KC * 512  # one bank's columns
            nc.sync.dma_start(v0_sb[:, :BK], v0_d[:, :BK])
            nc.sync.dma_start(h[:], h0_d[:])
            make_identity(nc, ident[:])
            for n in range(1, 4):
                nc.sync.dma_start(
                    v0_sb[:, n * BK : (n + 1) * BK], v0_d[:, n * BK : (n + 1) * BK]
                )
            for n in range(4):
                nc.sync.dma_start(
                    v_sb[:, n * BK : (n + 1) * BK], v_d[:, n * BK : (n + 1) * BK]
                )
            if has_bias:
                bias_sb = cpool.tile([P, 4 * HF], f32, tag="bias")
                nc.sync.dma_start(bias_sb[:], bias_d[:])

            # PE warm-up: transpose work depending only on the locally built
            # identity (not on any DMA) while the weight DMAs are in
            # flight. MUST be transpose-mode: a burst of 24 full-array
            # REGULAR matmuls here trips the P0 power state and the whole
            # chip gets latched at 2.0 GHz for the run (measured +100us).
            # Written into the trp0-tagged slot (reused from step 0 on).
            fil = warmpool.tile([BLOC, HF], f32, tag="fil", name="fil")
            wu = trpool.tile([P, P], f16, tag="trp0", name="wu")
            for i in range(24):
                nc.tensor.matmul(
                    wu[:],
                    ident[:],
                    ident[:],
                    is_transpose=True,
                    start=True,
                    stop=True,
                )

            HH2 = HF // 2
            for t in range(T):
                vsb = v0_sb if t == 0 else v_sb
                last = t == T - 1
                # one PSUM tile per gate bank: [r | hh | xh | z], each
                # [128, 256] folded (partitions 64f+b, cols = d % 256).
                # The z bank is SPLIT into two banks (columns 0:128 and
                # 128:256) computed sequentially, so sigmoid(z_L) and the
                # tt_L / h_newL chain run while the z_R half still streams
                # (PE-W + engine-R of one bank is fatal; separate banks
                # make the overlap legal).
                gb = [
                    gpool.tile([P, HF], f32, tag=f"g{n}", name=f"g{n}")
                    for n in range(3)
                ]
                gz = [
                    gpool.tile([P, HH2], f32, tag=f"gz{n}", name=f"gz{n}")
                    for n in range(2)
                ]

                def bank_mms(n):
                    for j, k in enumerate(KORDER):
                        for s2 in range(2):
                            nc.tensor.matmul(
                                gb[n][s2 * BLOC : (s2 + 1) * BLOC, :],
                                hT[:, CPOS[k] : CPOS[k] + BLOC],
                                vsb[
                                    :,
                                    n * BK + k * 512 + s2 * HF : n * BK
                                    + k * 512
                                    + (s2 + 1) * HF,
                                ],
                                start=(j == 0),
                                stop=(j == KC - 1),
                                tile_position=(0, s2 * BLOC),
                            )
                    if has_bias:
                        nc.vector.tensor_add(
                            gb[n][:], gb[n][:], bias_sb[:, n * HF : (n + 1) * HF]
                        )

                def zbank_mms(cc):
                    # z bank split into two COLUMN-HALF accumulation groups
                    # in separate PSUM banks: sigmoid(z_L) and the tt_L /
                    # h_newL chain overlap the z_R half's stream legally
                    # (PE-W + engine-R of one bank is fatal)
                    for j, k in enumerate(KORDER):
                        for s2 in range(2):
                            nc.tensor.matmul(
                                gz[cc][s2 * BLOC : (s2 + 1) * BLOC, :],
                                hT[:, CPOS[k] : CPOS[k] + BLOC],
                                vsb[
                                    :,
                                    3 * BK + k * 512 + s2 * HF + cc * HH2 : 3
                                    * BK
                                    + k * 512
                                    + s2 * HF
                                    + (cc + 1) * HH2,
                                ],
                                start=(j == 0),
                                stop=(j == KC - 1),
                                tile_position=(0, s2 * BLOC),
                            )
                    if has_bias:
                        nc.vector.tensor_add(
                            gz[cc][:],
                            gz[cc][:],
                            bias_sb[:, 3 * HF + cc * HH2 : 3 * HF + (cc + 1) * HH2],
                        )

                bank_mms(0)  # rpre
                r = wpool.tile([P, HF], f16, tag="r", name="r")
                nc.scalar.activation(r[:], gb[0][:], AF.Sigmoid)
                bank_mms(1)  # hh
                p = wpool.tile([P, HF], f16, tag="p", name="p")
                nc.vector.tensor_mul(p[:], r[:], gb[1][:])
                bank_mms(2)  # xh
                # q goes into the retired r-gate PSUM bank: ScalarE reads
                # PSUM faster than SBUF, so tanh starts sooner
                q = gb[0]
                nc.vector.tensor_add(q[:], p[:], gb[2][:])
                zbank_mms(0)  # zpre cols 0:128
                zbank_mms(1)  # zpre cols 128:256
                # HAM anchor: a half-array N=128 regular matmul right after
                # the stream keeps the activity monitor fed during the tail
                # (reads only consts, result dead; K=32 tiny ones are
                # invisible to HAM, full-K heavy ones trip the P0 power
                # state and downclock the whole chip 2.4 -> 2.0 GHz)
                nc.tensor.matmul(
                    fil[:, :], ident[:, :BLOC], hT[:],
                    start=True, stop=True,
                )
                hhat = wpool.tile([P, HF], f16, tag="hhat", name="hhat")
                nc.scalar.activation(hhat[:], q[:], AF.Tanh)
                # anchor on tanh: fires mid-tail before z_L
                nc.tensor.matmul(
                    fil[:, :], hhat[:, :BLOC], hT[:],
                    start=True, stop=True,
                )
                # z / tt / h_new split L|R so the L half (hT chunks 0,2)
                # clears the chain first. h_new = u*hhat + z*h with
                # u = sigmoid(-g_z) == 1-z: tt2 = z*h uses the OLD state so
                # it precomputes before tanh lands; only tt1 = u*hhat sits
                # on the tanh chain (drops the s = h - hhat hop)
                z = wpool.tile([P, HF], f16, tag="z", name="z")
                u = wpool.tile([P, HF], f16, tag="u", name="u")
                tt2 = wpool.tile([P, HF], f16, tag="t2", name="tt2")
                tt = wpool.tile([P, HF], f16, tag="t", name="tt")
                h_new = spool.tile([P, HF], f16, tag="h")
                HH = HF // 2
                trps = []
                if not last:
                    hT_new = spool.tile([P, HF], f16, tag="hT")
                for c in range(2):
                    cl = slice(c * HH, (c + 1) * HH)
                    nc.scalar.activation(z[:, cl], gz[c][:], AF.Sigmoid)
                    nc.scalar.activation(
                        u[:, cl], gz[c][:], AF.Sigmoid, scale=-1.0
                    )
                    nc.vector.tensor_mul(tt2[:, cl], z[:, cl], h[:, cl])
                    if c == 0 and not last:
                        # two more HAM anchors, tied to tail progress so
                        # the PE sees activity spread through the idle
                        # window (a warm 4096-cycle HAM window is only
                        # ~1.7us -- any longer PE-idle gap re-throttles)
                        nc.tensor.matmul(
                            fil[:, :], z[:, :BLOC], hT[:],
                            start=True, stop=True,
                        )
                        nc.tensor.matmul(
                            fil[:, :], tt2[:, :BLOC], hT[:],
                            start=True, stop=True,
                        )
                    nc.vector.tensor_mul(tt[:, cl], u[:, cl], hhat[:, cl])
                    nc.vector.tensor_add(h_new[:, cl], tt[:, cl], tt2[:, cl])
                    if not last:
                        # ONE full [128,128] transpose of h_new's half:
                        # out[c', 64f+b] = h[b, 256f+c'] which IS
                        # [hT chunk c | chunk c+2] — same layout the two
                        # [128,64] piece-transposes assembled, at half the
                        # serial PE time
                        trp = trpool.tile(
                            [P, P], f16, tag=f"trp{c}", name=f"trp{c}"
                        )
                        nc.tensor.matmul(
                            trp[:],
                            h_new[:, c * HH : (c + 1) * HH],
                            ident[:],
                            is_transpose=True,
                            start=True,
                            stop=True,
                        )
                        # chunk c (gates the next stream's k-order) on
                        # ACT as a narrow [128,64] copy; chunk c+2 moves
                        # to DVE after the loop
                        nc.scalar.copy(
                            hT_new[:, CPOS[c] : CPOS[c] + BLOC], trp[:, :BLOC]
                        )
                        trps.append(trp)
                if not last:
                    nc.vector.tensor_copy(
                        hT_new[:, CPOS[2] : CPOS[2] + BLOC], trps[0][:, BLOC:]
                    )
                    nc.vector.tensor_copy(
                        hT_new[:, CPOS[3] : CPOS[3] + BLOC], trps[1][:, BLOC:]
                    )
                    hT = hT_new

                nc.sync.dma_start(out_d[t, :, :], h_new[:])
                h = h_new

    nc.compile()
    return nc


def kernel(x, W, U, b):
    from concourse.bass_utils import run_bass_kernel_spmd

    x = np.asarray(x, dtype=np.float32)
    W = np.asarray(W, dtype=np.float32)
    U = np.asarray(U, dtype=np.float32)
    b = np.asarray(b, dtype=np.float32)

    V, V0, bias = _prepare_weights(W, U, b)
    has_bias = bool(np.any(bias != 0.0))
    v_dev = _dev_layout(V).astype(np.float16)
    v0_dev = _dev_layout(V0).astype(np.float16)

    key = ("gru16", has_bias)
    if key not in _CACHE:
        _CACHE[key] = _build(has_bias)
    nc = _CACHE[key]

    in_maps = []
    for i in range(NCORES):
        xs = x[i * BLOC : (i + 1) * BLOC].astype(np.float16)  # [64, 512]
        m = {
            "v0": v0_dev,
            "v": v_dev,
            # folded: h0[64f+b, c] = xs[b, 256f+c]
            "h0": np.ascontiguousarray(
                xs.reshape(BLOC, 2, HF).transpose(1, 0, 2).reshape(P, HF)
            ),
            # transposed: h0T[p, CPOS[k]+b] = xs[b, 128k+p]
            # (chunk column order 0,2,1,3)
            "h0T": np.ascontiguousarray(
                xs.reshape(BLOC, KC, P).transpose(2, 1, 0)[:, (0, 2, 1, 3), :]
                .reshape(P, KC * BLOC)
            ),
        }
        if has_bias:
            bf = bias.reshape(4, 2, HF).transpose(1, 0, 2)  # [f, n, c]
            m["bias"] = np.ascontiguousarray(
                np.broadcast_to(bf[:, None, :, :], (2, BLOC, 4, HF)).reshape(
                    P, 4 * HF
                )
            ).astype(np.float32)
        in_maps.append(m)

    # Rare (observed ~1/8 runs) HW race corrupts a few batch rows with NaNs
    # from t=0; the output is either fully clean or visibly NaN, so a
    # single nan-checked retry restores determinism of the result.
    for attempt in range(3):
        res = run_bass_kernel_spmd(
            nc, in_maps, core_ids=list(range(NCORES)), trace=TRACE, tmpdir=TMPDIR
        )
        LAST["exec_time_ns"] = res.exec_time_ns
        LAST["results"] = res
        outs = []
        for i in range(NCORES):
            o = res.results[i]["out"]  # [T, 128, 256] fp16
            o = np.asarray(o).reshape(T, 2, BLOC, HF)
            outs.append(o.transpose(2, 0, 1, 3).reshape(BLOC, T, D))
        full = np.concatenate(outs, axis=0).astype(np.float32)
        if not np.isnan(full).any():
            break
    return full


# revision 15
# speedup vs baseline: 1.1850x; 1.1850x over previous
"""Autoregressive GRU on 8 TRN2 NeuronCores — folded fp16 col-tiled design.

Data-parallel: batch B=512 split 64 rows/core; weights replicated; the T=128
sequential loop runs locally per core.

Math (Keras GRU, reset_after=True, gate order [z, r, h]), host-prefused:
  V  = [Wr+Ur | Uh | Wh | Wz+Uz]  [D, 4D]  (steps >= 1; step 0 uses V0)
  PSUM bank order [r | hh | xh | z]:
  r = sig(g_r); p = r*g_hh; q = p + g_xh; hhat = tanh(q); z = sig(g_z)
  h_new = hhat + z*(h - hhat)

Measured: 622291 ns HW exec (vs 906919 ns for the previous f32r/bf16
design on the same harness; 1077896 ns session-start baseline), rel err
1.93e-3 (was 9.6e-3). The HAM tail
anchors are five half-array K=128/M=64/N=256 matmuls tied to tail tiles
(ident/hhat/s/z/tt stationaries, hT moving): long enough to make the
tail's free-running HAM window look busy (the re-throttle decision lands
one window LATER, mid-next-stream — traced as streams starting warm at
106ns/pair then dropping to 207ns mid-stream), small enough (~+30
percent PE energy) to stay out of the P0 power state.

Perf structure vs the previous f32r design:
- FOLDED layout: every [64, 512] gate tile becomes [128, 256] by computing
  its two 256-col halves on PE column strips {0-63}/{64-127} via TWO
  CONCURRENT col-tiled matmuls (tile_position (0,0)/(0,64), same hT chunk
  stationary, different V halves). Microbenched: 16 N=512 f32r MMs =
  3796 ns/iter -> 16 concurrent N=256 fp16 pairs = 2195 ns/iter.
  Col tiling requires non-32-bit operands (walrus: f32r rejects dst
  partition 64; f32r×bf16 mixing unsupported) -> both operands fp16.
- fp16 (not bf16): 10 mantissa bits; values are small (|V|<0.2, |h|<~4) so
  no range risk. Numpy-emulated end-to-end rel err: fp16 everywhere 1.9e-3
  vs 9.8e-3 for the old f32r/bf16 design (gate is 2e-2).
- All DVE/ACT tail ops run [128, 256] (full 128 lanes) — 2x the old rate.
- hT rebuild: h_new = hhat + tt is computed first (split L/R halves),
  then ONE full [128,128] PE transpose per half: (h_new[:, 128c:+128])^T
  is exactly [hT chunk c | chunk c+2] in the folded layout (out[c',64f+b]
  = h[b, 256f+c']), full-partition stationary so no quadrant placement
  (transposes of [64, x] slices at partition base 64 hang the device —
  quadrant HW bug). Assembly: per-chunk copies (first chunk on ACT, the
  other on DVE). Next step's matmuls consume hT per 64-col chunk (subtile
  deps), k-order (0,2,1,3), so the stream restarts after the first chunk
  copy lands instead of after the full rebuild.
- trpL/trpR are separate PSUM banks, and each is read by a single copy
  that waits on both its writers (PE-W + engine-R same bank is fatal).
- Output is DMAed as fp16 straight from h_new (no f32 copy; host casts).
- Warm-up preamble of identity transposes flips the PE HAM clock gate to
  K=8/8 before step 0 (PE otherwise starts at 1.2 GHz).
"""

import numpy as np

B, D, T = 512, 512, 128
NCORES = 8
BLOC = B // NCORES  # 64
P = 128
KC = D // P  # 4 K-chunks
GW = 4 * D  # 2048 gate columns: [r | hh | xh | z]
HF = 256  # folded free size (D // 2)

# set by test harness to capture a profile; harmless when False
TRACE = False
TMPDIR = None
LAST = {}


def _prepare_weights(W, U, b):
    """Host-side fusion. Returns (V, V0, bias) in math layout."""
    Wz, Wr, Wh = W[:, :D], W[:, D : 2 * D], W[:, 2 * D :]
    Uz, Ur, Uh = U[:, :D], U[:, D : 2 * D], U[:, 2 * D :]
    V = np.concatenate([Wr + Ur, Uh, Wh, Wz + Uz], axis=1)  # [D, GW]
    V0 = np.concatenate([Ur, Uh, np.zeros_like(Wh), Uz], axis=1)
    b0, b1 = b[0], b[1]
    bias = np.concatenate(
        [b0[D : 2 * D] + b1[D : 2 * D], b1[2 * D :], b0[2 * D :], b0[:D] + b1[:D]]
    )  # [GW], order [r | hh | xh | z]
    return V, V0, bias


def _dev_layout(V):
    # bank-major: V_dev[p, n*(KC*512) + k*512 + c] = V[k*128 + p, n*512 + c]
    # so each gate bank's weights are one contiguous 512KB block and the
    # first stream can start as soon as bank r's DMA lands
    A = V.reshape(KC, P, 4, 512)
    return np.ascontiguousarray(A.transpose(1, 2, 0, 3).reshape(P, KC * GW))


_CACHE = {}

KORDER = (0, 2, 1, 3)  # consume hT L-half chunks (0,2) first
# hT column layout: chunk k lives at cols [CPOS[k], CPOS[k]+64)
CPOS = {0: 0, 2: 64, 1: 128, 3: 192}


def _build(has_bias: bool):
    import concourse.mybir as mybir
    import concourse.tile as tile
    from concourse import bacc
    from concourse.masks import make_identity

    f32 = mybir.dt.float32
    f16 = mybir.dt.float16
    AF = mybir.ActivationFunctionType

    nc = bacc.Bacc(
        "TRN2", target_bir_lowering=False, debug=False, num_devices=NCORES
    )
    v0_d = nc.dram_tensor("v0", [P, KC * GW], f16, kind="ExternalInput").ap()
    v_d = nc.dram_tensor("v", [P, KC * GW], f16, kind="ExternalInput").ap()
    h0_d = nc.dram_tensor("h0", [P, HF], f16, kind="ExternalInput").ap()
    h0T_d = nc.dram_tensor("h0T", [P, HF], f16, kind="ExternalInput").ap()
    if has_bias:
        bias_d = nc.dram_tensor("bias", [P, 4 * HF], f32, kind="ExternalInput").ap()
    out_d = nc.dram_tensor("out", [T, P, HF], f16, kind="ExternalOutput").ap()

    with tile.TileContext(nc) as tc:
        with (
            tc.tile_pool(name="const", bufs=1) as cpool,
            tc.tile_pool(name="state", bufs=2) as spool,
            tc.tile_pool(name="work", bufs=3) as wpool,
            tc.tile_pool(name="gates", bufs=1, space="PSUM") as gpool,
            tc.tile_pool(name="trp", bufs=1, space="PSUM") as trpool,
            tc.tile_pool(name="warm", bufs=1, space="PSUM") as warmpool,
        ):
            v0_sb = cpool.tile([P, KC * GW], f16, tag="v0")
            v_sb = cpool.tile([P, KC * GW], f16, tag="v")
            ident = cpool.tile([P, P], f16, tag="ident")
            h = spool.tile([P, HF], f16, tag="h")
            hT = spool.tile([P, HF], f16, tag="hT")
            nc.sync.dma_start(hT[:], h0T_d[:])
            BK = KC * 512  # one bank's columns
            nc.sync.dma_start(v0_sb[:, :BK], v0_d[:, :BK])
            nc.sync.dma_start(h[:], h0_d[:])
            make_identity(nc, ident[:])
            for n in range(1, 4):
                nc.sync.dma_start(
                    v0_sb[:, n * BK : (n + 1) * BK], v0_d[:, n * BK : (n + 1) * BK]
                )
            for n in range(4):
                nc.sync.dma_start(
                    v_sb[:, n * BK : (n + 1) * BK], v_d[:, n * BK : (n + 1) * BK]
                )
            if has_bias:
                bias_sb = cpool.tile([P, 4 * HF], f32, tag="bias")
                nc.sync.dma_start(bias_sb[:], bias_d[:])

            # PE warm-up: transpose work depending only on the locally built
            # identity (not on any DMA) while the weight DMAs are in
            # flight. MUST be transpose-mode: a burst of 24 full-array
            # REGULAR matmuls here trips the P0 power state and the whole
            # chip gets latched at 2.0 GHz for the run (measured +100us).
            # Written into the trp0-tagged slot (reused from step 0 on).
            fil = warmpool.tile([BLOC, HF], f32, tag="fil", name="fil")
            wu = trpool.tile([P, P], f16, tag="trp0", name="wu")
            for i in range(24):
                nc.tensor.matmul(
                    wu[:],
                    ident[:],
                    ident[:],
                    is_transpose=True,
                    start=True,
                    stop=True,
                )

            HH2 = HF // 2
            for t in range(T):
                vsb = v0_sb if t == 0 else v_sb
                last = t == T - 1
                # one PSUM tile per gate bank: [r | hh | xh | z], each
                # [128, 256] folded (partitions 64f+b, cols = d % 256).
                # The z bank is SPLIT into two banks (columns 0:128 and
                # 128:256) computed sequentially, so sigmoid(z_L) and the
                # tt_L / h_newL chain run while the z_R half still streams
                # (PE-W + engine-R of one bank is fatal; separate banks
                # make the overlap legal).
                gb = [
                    gpool.tile([P, HF], f32, tag=f"g{n}", name=f"g{n}")
                    for n in range(3)
                ]
                gz = [
                    gpool.tile([P, HH2], f32, tag=f"gz{n}", name=f"gz{n}")
                    for n in range(2)
                ]

                def bank_mms(n):
                    for j, k in enumerate(KORDER):
                        for s2 in range(2):
                            nc.tensor.matmul(
                                gb[n][s2 * BLOC : (s2 + 1) * BLOC, :],
                                hT[:, CPOS[k] : CPOS[k] + BLOC],
                                vsb[
                                    :,
                                    n * BK + k * 512 + s2 * HF : n * BK
                                    + k * 512
                                    + (s2 + 1) * HF,
                                ],
                                start=(j == 0),
                                stop=(j == KC - 1),
                                tile_position=(0, s2 * BLOC),
                            )
                    if has_bias:
                        nc.vector.tensor_add(
                            gb[n][:], gb[n][:], bias_sb[:, n * HF : (n + 1) * HF]
                        )

                def zbank_mms(cc):
                    # z bank split into two COLUMN-HALF accumulation groups
                    # in separate PSUM banks: sigmoid(z_L) and the tt_L /
                    # h_newL chain overlap the z_R half's stream legally
                    # (PE-W + engine-R of one bank is fatal)
                    for j, k in enumerate(KORDER):
                        for s2 in range(2):
                            nc.tensor.matmul(
                                gz[cc][s2 * BLOC : (s2 + 1) * BLOC, :],
                                hT[:, CPOS[k] : CPOS[k] + BLOC],
                                vsb[
                                    :,
                                    3 * BK + k * 512 + s2 * HF + cc * HH2 : 3
                                    * BK
                                    + k * 512
                                    + s2 * HF
                                    + (cc + 1) * HH2,
                                ],
                                start=(j == 0),
                                stop=(j == KC - 1),
                                tile_position=(0, s2 * BLOC),
                            )
                    if has_bias:
                        nc.vector.tensor_add(
                            gz[cc][:],
                            gz[cc][:],
                            bias_sb[:, 3 * HF + cc * HH2 : 3 * HF + (cc + 1) * HH2],
                        )

                bank_mms(0)  # rpre
                r = wpool.tile([P, HF], f16, tag="r", name="r")
                nc.scalar.activation(r[:], gb[0][:], AF.Sigmoid)
                bank_mms(1)  # hh
                p = wpool.tile([P, HF], f16, tag="p", name="p")
                nc.vector.tensor_mul(p[:], r[:], gb[1][:])
                bank_mms(2)  # xh
                # q goes into the retired r-gate PSUM bank: ScalarE reads
                # PSUM faster than SBUF, so tanh starts sooner
                q = gb[0]
                nc.vector.tensor_add(q[:], p[:], gb[2][:])
                zbank_mms(0)  # zpre cols 0:128
                zbank_mms(1)  # zpre cols 128:256
                # HAM anchor: a half-array N=128 regular matmul right after
                # the stream keeps the activity monitor fed during the tail
                # (reads only consts, result dead; K=32 tiny ones are
                # invisible to HAM, full-K heavy ones trip the P0 power
                # state and downclock the whole chip 2.4 -> 2.0 GHz)
                nc.tensor.matmul(
                    fil[:, :], ident[:, :BLOC], hT[:],
                    start=True, stop=True,
                )
                hhat = wpool.tile([P, HF], f16, tag="hhat", name="hhat")
                nc.scalar.activation(hhat[:], q[:], AF.Tanh)
                # anchor on tanh: fires mid-tail before z_L
                nc.tensor.matmul(
                    fil[:, :], hhat[:, :BLOC], hT[:],
                    start=True, stop=True,
                )
                s = wpool.tile([P, HF], f16, tag="s", name="s")
                nc.vector.tensor_sub(s[:], h[:], hhat[:])
                nc.tensor.matmul(
                    fil[:, :], s[:, :BLOC], hT[:],
                    start=True, stop=True,
                )
                # z / tt / h_new split L|R so the L half (hT chunks 0,2)
                # clears the chain first
                z = wpool.tile([P, HF], f16, tag="z", name="z")
                tt = wpool.tile([P, HF], f16, tag="t", name="tt")
                h_new = spool.tile([P, HF], f16, tag="h")
                HH = HF // 2
                trps = []
                if not last:
                    hT_new = spool.tile([P, HF], f16, tag="hT")
                for c in range(2):
                    cl = slice(c * HH, (c + 1) * HH)
                    nc.scalar.activation(z[:, cl], gz[c][:], AF.Sigmoid)
                    nc.vector.tensor_mul(tt[:, cl], z[:, cl], s[:, cl])
                    if c == 0 and not last:
                        # two more HAM anchors, tied to tail progress so
                        # the PE sees activity spread through the idle
                        # window (a warm 4096-cycle HAM window is only
                        # ~1.7us -- any longer PE-idle gap re-throttles)
                        nc.tensor.matmul(
                            fil[:, :], z[:, :BLOC], hT[:],
                            start=True, stop=True,
                        )
                        nc.tensor.matmul(
                            fil[:, :], tt[:, :BLOC], hT[:],
                            start=True, stop=True,
                        )
                    nc.vector.tensor_add(h_new[:, cl], hhat[:, cl], tt[:, cl])
                    if not last:
                        # ONE full [128,128] transpose of h_new's half:
                        # out[c', 64f+b] = h[b, 256f+c'] which IS
                        # [hT chunk c | chunk c+2] — same layout the two
                        # [128,64] piece-transposes assembled, at half the
                        # serial PE time
                        trp = trpool.tile(
                            [P, P], f16, tag=f"trp{c}", name=f"trp{c}"
                        )
                        nc.tensor.matmul(
                            trp[:],
                            h_new[:, c * HH : (c + 1) * HH],
                            ident[:],
                            is_transpose=True,
                            start=True,
                            stop=True,
                        )
                        # chunk c (gates the next stream's k-order) on
                        # ACT as a narrow [128,64] copy; chunk c+2 moves
                        # to DVE after the loop
                        nc.scalar.copy(
                            hT_new[:, CPOS[c] : CPOS[c] + BLOC], trp[:, :BLOC]
                        )
                        trps.append(trp)
                if not last:
                    nc.vector.tensor_copy(
                        hT_new[:, CPOS[2] : CPOS[2] + BLOC], trps[0][:, BLOC:]
                    )
                    nc.vector.tensor_copy(
                        hT_new[:, CPOS[3] : CPOS[3] + BLOC], trps[1][:, BLOC:]
                    )
                    hT = hT_new

                nc.sync.dma_start(out_d[t, :, :], h_new[:])
                h = h_new

    nc.compile()
    return nc


def kernel(x, W, U, b):
    from concourse.bass_utils import run_bass_kernel_spmd

    x = np.asarray(x, dtype=np.float32)
    W = np.asarray(W, dtype=np.float32)
    U = np.asarray(U, dtype=np.float32)
    b = np.asarray(b, dtype=np.float32)

    V, V0, bias = _prepare_weights(W, U, b)
    has_bias = bool(np.any(bias != 0.0))
    v_dev = _dev_layout(V).astype(np.float16)
    v0_dev = _dev_layout(V0).astype(np.float16)

    key = ("gru16", has_bias)
    if key not in _CACHE:
        _CACHE[key] = _build(has_bias)
    nc = _CACHE[key]

    in_maps = []
    for i in range(NCORES):
        xs = x[i * BLOC : (i + 1) * BLOC].astype(np.float16)  # [64, 512]
        m = {
            "v0": v0_dev,
            "v": v_dev,
            # folded: h0[64f+b, c] = xs[b, 256f+c]
            "h0": np.ascontiguousarray(
                xs.reshape(BLOC, 2, HF).transpose(1, 0, 2).reshape(P, HF)
            ),
            # transposed: h0T[p, CPOS[k]+b] = xs[b, 128k+p]
            # (chunk column order 0,2,1,3)
            "h0T": np.ascontiguousarray(
                xs.reshape(BLOC, KC, P).transpose(2, 1, 0)[:, (0, 2, 1, 3), :]
                .reshape(P, KC * BLOC)
            ),
        }
        if has_bias:
            bf = bias.reshape(4, 2, HF).transpose(1, 0, 2)  # [f, n, c]
            m["bias"] = np.ascontiguousarray(
                np.broadcast_to(bf[:, None, :, :], (2, BLOC, 4, HF)).reshape(
                    P, 4 * HF
                )
            ).astype(np.float32)
        in_maps.append(m)

    # Rare (observed ~1/8 runs) HW race corrupts a few batch rows with NaNs
    # from t=0; the output is either fully clean or visibly NaN, so a
    # single nan-checked retry restores determinism of the result.
    for attempt in range(3):
        res = run_bass_kernel_spmd(
            nc, in_maps, core_ids=list(range(NCORES)), trace=TRACE, tmpdir=TMPDIR
        )
        LAST["exec_time_ns"] = res.exec_time_ns
        LAST["results"] = res
        outs = []
        for i in range(NCORES):
            o = res.results[i]["out"]  # [T, 128, 256] fp16
            o = np.asarray(o).reshape(T, 2, BLOC, HF)
            outs.append(o.transpose(2, 0, 1, 3).reshape(BLOC, T, D))
        full = np.concatenate(outs, axis=0).astype(np.float32)
        if not np.isnan(full).any():
            break
    return full
